# revision 1
# baseline (speedup 1.0000x reference)
"""Trainium2 Bass kernel for nn_ControlPolicy (T=4096, B=256, N=64, K=2, A=16).

Sharding: data-parallel over the batch axis B across 8 NeuronCores (32 rows
per core); tiny parameters replicated.

Per-core algorithm:
  All linear recurrences (z low-pass filter, phase integrator, i_s
  integrator) run at line rate on the DVE tensor_tensor_scan instruction with
  time on the free axis.  With sum_k softmax(w)_k == 1 and uniform kp/ki/kd
  (np.full in setup_inputs), the PID algebra collapses so the only truly
  sequential computation is a small nonlinear recurrence in (a, D):

      u = C_t - kappa*D ; h = tanh(u) ; q = h - a ; r = tanh(s2*q)
      a' = a + rate*r   ; D' = lam2*D + rate*r - beta*q

  where C_t is bulk-precomputable.  This is evaluated with an overlap-save
  chunked sweep: T is cut into chunks of R=64 steps, every chunk starts from
  zero state W=32 steps early, and the contraction of the per-step map washes
  out the wrong start (validated to ~1e-5 abs against the jax reference).
  All 64 chunks advance simultaneously inside each DVE/ACT instruction.

Streaming phase per superblock of 512 t:
  layernorm (reduce stats + broadcast apply) -> PE transpose to [(b,n),t] ->
  PE projections (e-features per k, omega/gate smalls) -> alpha-filter scans
  -> smalls pipeline (omega, phi scan+wrap, sin/cos, gate sigmoid) ->
  phase-feature matmuls -> e assembly -> integrator scan -> C assembly.
The C and a time-buffers share one SBUF allocation: the sweep's a-writes land
exactly on C columns already consumed in the same step.
"""
import math
import numpy as np
from contextlib import ExitStack

import concourse.bass as bass
import concourse.bacc as bacc
import concourse.tile as tile
from concourse import mybir
from concourse.bass_utils import run_bass_kernel_spmd
from concourse.masks import make_identity

F32 = mybir.dt.float32
F16 = mybir.dt.float16
OP = mybir.AluOpType
AF = mybir.ActivationFunctionType
AX = mybir.AxisListType

T_FULL = 4096
B_FULL = 256
N = 64
K = 2
A = 16
NCORES = 8
BL = B_FULL // NCORES          # 32
LN_EPS = 1e-5
TWO_PI = float(np.float32(2.0 * np.pi))

R = 64                          # sweep chunk length
W = 32                          # sweep warm-up
NSUP = 4                        # supersets of 8 b-rows
NPAIR = BL // 2                 # 16


def _sigmoid(x): return 1.0 / (1.0 + math.exp(-x))
def _softplus(x): return math.log1p(math.exp(x))


def _coeffs(inputs):
    f = lambda k: float(np.asarray(inputs[k], np.float64))
    alpha = _sigmoid(f("filter_alpha_logit"))
    leak = _sigmoid(f("int_leak_logit"))
    beta = _sigmoid(f("act_beta_logit"))
    rate = 0.25 * _sigmoid(f("rate_limit_raw"))
    aw = _softplus(f("aw_gain_raw"))
    omega_base = _softplus(f("phase_omega_raw")) + 0.001

    kp_a = np.log1p(np.exp(np.asarray(inputs["kp_raw"], np.float64)))
    ki_a = np.log1p(np.exp(np.asarray(inputs["ki_raw"], np.float64)))
    kd_a = np.log1p(np.exp(np.asarray(inputs["kd_raw"], np.float64)))
    for nm, arr in (("kp", kp_a), ("ki", ki_a), ("kd", kd_a)):
        assert np.allclose(arr, arr.flat[0], rtol=1e-12), f"{nm} not uniform"
    kp, ki, kd = float(kp_a.flat[0]), float(ki_a.flat[0]), float(kd_a.flat[0])

    lam2 = 1.0 - leak
    c1 = kp + kd
    co = dict(
        alpha=alpha, lam=1.0 - alpha, lam2=lam2, beta=beta, rate=rate,
        omega_base=omega_base, c1=c1, kappa=ki * aw * lam2,
        s2=beta / (rate + 1e-6), ki_c1=ki / c1, kd_c1=kd / c1,
    )

    ln_w = np.asarray(inputs["ln_w"], np.float64)
    ln_b = np.asarray(inputs["ln_b"], np.float64)
    w_state = np.asarray(inputs["w_state"], np.float64)
    w_phase = np.asarray(inputs["w_phase"], np.float64)
    b_err = np.asarray(inputs["b_err"], np.float64)
    gate_w = np.asarray(inputs["gate_w"], np.float64)
    gate_b = np.asarray(inputs["gate_b"], np.float64)
    pos = np.asarray(inputs["phase_omega_state"], np.float64)

    # e-feature projection lhsT [128, K*4*128]: accumulation-chain blocks.
    # Block (k, pw) maps pair pw's partitions (bw2, n64) onto out rows
    # (b8, a16) = 16*(2*pw+bw) + a of the k psum tile.
    we = np.zeros((128, K * 4 * 128), np.float64)
    for k in range(K):
        for pw in range(4):
            base = (k * 4 + pw) * 128
            for bw in range(2):
                for a in range(A):
                    we[bw * 64:(bw + 1) * 64,
                       base + 16 * (2 * pw + bw) + a] = (
                        c1 * alpha * w_state[k, a, :] * ln_w)
    # smalls projection lhsT [128, 16*64]: accumulation-chain blocks per pair.
    # Out rows type-major: 32*c + b with c in {pos, gdiff = g0 - g1}.
    wsm_rows = [alpha * pos * ln_w, alpha * (gate_w[0] - gate_w[1]) * ln_w]
    wsm = np.zeros((128, 16 * 64), np.float64)
    for p in range(16):
        for bw in range(2):
            for c in range(2):
                wsm[bw * 64:(bw + 1) * 64,
                    p * 64 + 32 * c + 2 * p + bw] = wsm_rows[c]
    # projection biases from ln_b (zero in practice -> matmuls skipped)
    be = np.stack([c1 * alpha * (w_state[k] @ ln_b) for k in range(K)])
    bsm = np.array([alpha * (pos @ ln_b),
                    alpha * ((gate_w[0] - gate_w[1]) @ ln_b)])
    co["has_proj_bias"] = bool(np.any(be != 0) or np.any(bsm != 0))
    ebias = np.zeros((1, 128 * K), np.float64)
    for k in range(K):
        ebias[0, 128 * k:128 * (k + 1)] = np.tile(be[k], 8)
    smbias = np.repeat(bsm, 32).reshape(1, 64)

    eph = np.zeros((64, NSUP * K * 128), np.float64)
    for s in range(NSUP):
        for k in range(K):
            base = (s * K + k) * 128
            for bb in range(8):
                for a in range(A):
                    eph[8 * s + bb, base + 16 * bb + a] = c1 * w_phase[k, a, 0]
                    eph[32 + 8 * s + bb, base + 16 * bb + a] = c1 * w_phase[k, a, 1]
    w0b = np.zeros((32, NSUP * 128), np.float64)
    for s in range(NSUP):
        for bb in range(8):
            for a in range(A):
                w0b[8 * s + bb, s * 128 + 16 * bb + a] = 1.0
    berr = np.zeros((128, K), np.float64)
    for k in range(K):
        berr[:, k] = np.tile(c1 * b_err[k], 8)

    co["dgb"] = float(gate_b[0] - gate_b[1])
    consts = {nm: arr.astype(np.float16) for nm, arr in dict(
        c_we=we, c_wsm=wsm, c_eph=eph, c_w0b=w0b).items()}
    consts.update({nm: arr.astype(np.float32) for nm, arr in dict(
        c_berr=berr, c_ebias=ebias, c_smbias=smbias).items()})
    return co, consts


def build_program(nc, co, t_total=T_FULL, ts=512, debug_taps=False):
    nsb = t_total // ts
    nch = t_total // R
    ncol = R * (nch + 1)
    ntc = ts // 128                 # 128-row t-chunks per superblock

    x_in = nc.dram_tensor("x", [t_total, BL, N], F32, kind="ExternalInput").ap()
    out_d = nc.dram_tensor("out", [t_total, BL, A], F32, kind="ExternalOutput").ap()
    shapes = dict(c_we=(128, K * 4 * 128), c_wsm=(128, 16 * 64),
                  c_eph=(64, NSUP * K * 128), c_w0b=(32, NSUP * 128),
                  c_berr=(128, K), c_ebias=(1, 128 * K), c_smbias=(1, 64))
    half = {"c_we", "c_wsm", "c_eph", "c_w0b"}
    cw = {nm: nc.dram_tensor(nm, list(sh), F16 if nm in half else F32,
                             kind="ExternalInput").ap()
          for nm, sh in shapes.items()}
    taps = {}
    if debug_taps:
        for spec in [("d_xn", (128, BL * N), F16), ("d_ez", (128, t_total)),
                     ("d_eh", (K, 128, t_total)), ("d_si", (K, 128, t_total)),
                     ("d_yk", (K, 128, t_total)), ("d_w0", (32, t_total), F16),
                     ("d_sc", (64, t_total), F16), ("d_C", (128, t_total))]:
            nm, sh = spec[0], spec[1]
            dt_ = spec[2] if len(spec) > 2 else F32
            taps[nm] = nc.dram_tensor(nm, list(sh), dt_, kind="ExternalOutput").ap()

    with tile.TileContext(nc) as tc, ExitStack() as top:
        consts = top.enter_context(tc.tile_pool(name="consts", bufs=1))
        carry = top.enter_context(tc.tile_pool(name="carry", bufs=1))
        bigp = top.enter_context(tc.tile_pool(name="big", bufs=1))

        ct = {}
        for nm, ap in cw.items():
            t = consts.tile(list(ap.shape), ap.dtype, tag=nm)
            nc.sync.dma_start(out=t, in_=ap)
            ct[nm] = t
        ident = consts.tile([128, 128], F32)
        make_identity(nc, ident)
        ident_h = consts.tile([128, 128], F16)
        nc.vector.tensor_copy(out=ident_h, in_=ident)
        eps_col = consts.tile([128, 1], F32); nc.vector.memset(eps_col, LN_EPS)
        halfpi = consts.tile([128, 1], F32); nc.vector.memset(halfpi, math.pi / 2)
        lam_col = consts.tile([128, 1], F32); nc.vector.memset(lam_col, co["lam"])
        lam2_col = consts.tile([128, 1], F32); nc.vector.memset(lam2_col, co["lam2"])
        one_col = consts.tile([128, 1], F32); nc.vector.memset(one_col, 1.0)
        dgb_col = consts.tile([64, 1], F32); nc.vector.memset(dgb_col, 0.5 * co["dgb"])
        ones_row = consts.tile([1, ts], F32); nc.vector.memset(ones_row, 1.0)

        c_ez = carry.tile([128, 2 * NSUP], F32); nc.vector.memset(c_ez, 0.0)
        c_si = carry.tile([128, 2 * NSUP], F32); nc.vector.memset(c_si, 0.0)
        c_ep = carry.tile([128, 2 * NSUP], F32); nc.vector.memset(c_ep, 0.0)
        c_sm = carry.tile([64, 1], F32); nc.vector.memset(c_sm, 0.0)
        c_phi = carry.tile([32, 1], F32); nc.vector.memset(c_phi, 0.0)

        # C time buffer (head zero-padded for chunk-0 warm-up), fp16
        ca = bigp.tile([128, NSUP, ncol], F16)
        for g in range(NSUP):
            nc.vector.memset(ca[:, g, 0:R], 0.0)

        # ================= streaming phase =================
        with ExitStack() as pha:
            p_x = pha.enter_context(tc.tile_pool(name="px", bufs=4))
            p_xsq = pha.enter_context(tc.tile_pool(name="pxsq", bufs=2))
            p_xh = pha.enter_context(tc.tile_pool(name="pxh", bufs=4))
            p_st = pha.enter_context(tc.tile_pool(name="pst", bufs=1))
            p_xnt = pha.enter_context(tc.tile_pool(name="pxnt", bufs=6))
            p_ez = pha.enter_context(tc.tile_pool(name="pez", bufs=9))
            p_eh = pha.enter_context(tc.tile_pool(name="peh", bufs=3))
            p_si = pha.enter_context(tc.tile_pool(name="psi", bufs=2))
            p_yk = pha.enter_context(tc.tile_pool(name="pyk", bufs=3))
            p_dy = pha.enter_context(tc.tile_pool(name="pdy", bufs=2))
            p_sm = pha.enter_context(tc.tile_pool(name="psm", bufs=1))
            ps_tp = pha.enter_context(tc.tile_pool(name="pstp", bufs=2, space="PSUM"))
            ps_ep = pha.enter_context(tc.tile_pool(name="psep", bufs=2, space="PSUM"))
            ps_eh = pha.enter_context(tc.tile_pool(name="pseh", bufs=2, space="PSUM"))
            ps_w0 = pha.enter_context(tc.tile_pool(name="psw0", bufs=1, space="PSUM"))
            ps_sm = pha.enter_context(tc.tile_pool(name="pssm", bufs=1, space="PSUM"))

            for sb in range(nsb):
                t0 = sb * ts
                # ---- load + layernorm (layout [t, (b,n)]) ----
                xch = []
                for c in range(ntc):
                    xt = p_x.tile([128, BL * N], F32, tag="xch")
                    nc.sync.dma_start(
                        out=xt,
                        in_=x_in[t0 + 128 * c: t0 + 128 * (c + 1)]
                        .rearrange("t b n -> t (b n)"))
                    xch.append(xt)
                # stats for all chunks batched into [128, ntc, BL] tiles
                s1c = p_st.tile([128, ntc, BL], F32, tag="s1c")
                s2c = p_st.tile([128, ntc, BL], F32, tag="s2c")
                for c in range(ntc):
                    xt = xch[c]
                    x3 = xt.rearrange("t (b n) -> t b n", b=BL)
                    xsq = p_xsq.tile([128, BL * N], F32, tag="xsq")
                    nc.scalar.activation(out=xsq, in_=xt, func=AF.Square)
                    nc.vector.tensor_reduce(out=s1c[:, c, :], in_=x3, axis=AX.X,
                                            op=OP.add)
                    nc.vector.tensor_reduce(
                        out=s2c[:, c, :],
                        in_=xsq.rearrange("t (b n) -> t b n", b=BL),
                        axis=AX.X, op=OP.add)
                # mu, var+eps, invstd = fast-rsqrt + 2 Newton steps (all DVE)
                mu = p_st.tile([128, ntc, BL], F32, tag="mu")
                nc.vector.tensor_scalar(out=mu, in0=s1c, scalar1=1.0 / N,
                                        scalar2=None, op0=OP.mult)
                msq = p_st.tile([128, ntc, BL], F32, tag="msq")
                nc.vector.tensor_tensor(out=msq, in0=mu, in1=mu, op=OP.mult)
                vpe = s2c
                nc.vector.scalar_tensor_tensor(
                    out=vpe, in0=s2c, scalar=1.0 / N, in1=msq,
                    op0=OP.mult, op1=OP.subtract)
                nc.vector.tensor_scalar(out=vpe, in0=vpe, scalar1=LN_EPS,
                                        scalar2=None, op0=OP.add)
                vh = msq
                nc.vector.tensor_scalar(out=vh, in0=vpe, scalar1=0.5,
                                        scalar2=None, op0=OP.mult)
                inv = p_st.tile([128, ntc, BL], F32, tag="inv")
                ivi = inv.bitcast(mybir.dt.int32)
                nc.vector.tensor_scalar(out=ivi, in0=vpe.bitcast(mybir.dt.int32),
                                        scalar1=1, scalar2=None,
                                        op0=OP.arith_shift_right)
                nc.vector.tensor_scalar(out=ivi, in0=ivi, scalar1=-1,
                                        scalar2=0x5f3759df, op0=OP.mult,
                                        op1=OP.add)
                for _ in range(2):
                    yy = p_st.tile([128, ntc, BL], F32, tag="yy")
                    nc.vector.tensor_tensor(out=yy, in0=inv, in1=inv, op=OP.mult)
                    nc.vector.tensor_tensor(out=yy, in0=yy, in1=vh, op=OP.mult)
                    nc.vector.tensor_scalar(out=yy, in0=yy, scalar1=-1.0,
                                            scalar2=1.5, op0=OP.mult, op1=OP.add)
                    nc.vector.tensor_tensor(out=inv, in0=inv, in1=yy, op=OP.mult)
                xnh = []
                for c in range(ntc):
                    xt = xch[c]
                    x3 = xt.rearrange("t (b n) -> t b n", b=BL)
                    nc.gpsimd.tensor_tensor(
                        out=x3, in0=x3,
                        in1=mu[:, c, :, None].broadcast_to([128, BL, N]),
                        op=OP.subtract)
                    xh = p_xh.tile([128, BL * N], F16, tag="xnh")
                    nc.gpsimd.tensor_tensor(
                        out=xh.rearrange("t (b n) -> t b n", b=BL), in0=x3,
                        in1=inv[:, c, :, None].broadcast_to([128, BL, N]),
                        op=OP.mult)
                    xnh.append(xh)
                    if debug_taps and sb == 0 and c == 0:
                        nc.sync.dma_start(out=taps["d_xn"], in_=xh)

                # ---- transpose + projections + filter scans ----
                smp = ps_sm.tile([64, ts], F32, tag="smp")
                ez_t = {}
                for s in range(NSUP):
                    eps_ = {}
                    for k in range(K):
                        epk = ps_ep.tile([128, ts], F32, tag="epre")
                        eps_[k] = epk
                    for pw in range(4):
                        p = 4 * s + pw
                        tpt = ps_tp.tile([128, ts], F16, tag="tp")
                        for c in range(ntc):
                            nc.tensor.transpose(
                                tpt[:, 128 * c: 128 * (c + 1)],
                                xnh[c][:, 128 * p: 128 * (p + 1)], ident_h)
                        xT = p_xnt.tile([128, ts], F16, tag="xnT")
                        nc.scalar.copy(out=xT, in_=tpt)
                        for k in range(K):
                            nc.tensor.matmul(
                                out=eps_[k],
                                lhsT=ct["c_we"][:, (k * 4 + pw) * 128:
                                                (k * 4 + pw + 1) * 128],
                                rhs=xT, start=(pw == 0), stop=(pw == 3))
                        nc.tensor.matmul(out=smp,
                                         lhsT=ct["c_wsm"][:, p * 64:(p + 1) * 64],
                                         rhs=xT, start=(p == 0), stop=(p == 15))
                    for k in range(K):
                        if co["has_proj_bias"]:
                            nc.tensor.matmul(
                                out=eps_[k],
                                lhsT=ct["c_ebias"][:, 128 * k:128 * (k + 1)],
                                rhs=ones_row, start=False, stop=True,
                                skip_group_check=True)
                        ez = p_ez.tile([128, ts], F32, tag="ez")
                        sk = 2 * s + k
                        nc.vector.tensor_tensor_scan(
                            out=ez, data0=lam_col.broadcast_to([128, ts]),
                            data1=eps_[k], initial=c_ez[:, sk:sk + 1],
                            op0=OP.mult, op1=OP.add)
                        nc.gpsimd.tensor_copy(out=c_ez[:, sk:sk + 1],
                                              in_=ez[:, ts - 1:ts])
                        if debug_taps and s == 0 and k == 0:
                            nc.sync.dma_start(out=taps["d_ez"][:, t0:t0 + ts],
                                              in_=ez)
                        ez_t[(s, k)] = ez
                if co["has_proj_bias"]:
                    nc.tensor.matmul(out=smp, lhsT=ct["c_smbias"], rhs=ones_row,
                                     start=False, stop=True,
                                     skip_group_check=True)

                # ---- smalls pipeline ----
                sms = p_sm.tile([64, ts], F32, tag="sms")
                nc.vector.tensor_tensor_scan(
                    out=sms, data0=lam_col[0:64].broadcast_to([64, ts]),
                    data1=smp, initial=c_sm, op0=OP.mult, op1=OP.add)
                nc.gpsimd.tensor_copy(out=c_sm, in_=sms[:, ts - 1:ts])
                sigp = p_sm.tile([32, ts], F32, tag="sigp")
                nc.scalar.activation(out=sigp, in_=sms[0:32, :], func=AF.Tanh)
                om = p_sm.tile([32, ts], F32, tag="om")
                nc.vector.tensor_scalar(out=om, in0=sigp, scalar1=0.02,
                                        scalar2=co["omega_base"],
                                        op0=OP.mult, op1=OP.add)
                nc.vector.tensor_scalar(out=om, in0=om, scalar1=1.0,
                                        scalar2=0.001, op0=OP.min, op1=OP.max)
                phr = p_sm.tile([32, ts], F32, tag="phr")
                nc.vector.tensor_tensor_scan(
                    out=phr, data0=one_col[0:32].broadcast_to([32, ts]),
                    data1=om, initial=c_phi, op0=OP.mult, op1=OP.add)
                # wrap to [-pi, pi] via round-to-nearest int convert:
                #   w = x - 2*pi*round(x/(2*pi)); sin periodic, so sin(w)=sin(x)
                sc = p_sm.tile([64, ts], F16, tag="sc")
                wf = p_sm.tile([32, ts], F32, tag="wf")
                wi = p_sm.tile([32, ts], mybir.dt.int32, tag="wi")
                wrap = p_sm.tile([32, ts], F32, tag="wrap")
                nc.vector.tensor_scalar(out=wf, in0=phr, scalar1=(1.0 / (2.0 * math.pi)),
                                        scalar2=None, op0=OP.mult)
                nc.vector.tensor_copy(out=wi, in_=wf)
                nc.vector.tensor_copy(out=wf, in_=wi)
                nc.vector.scalar_tensor_tensor(out=wrap, in0=wf, scalar=-TWO_PI,
                                               in1=phr, op0=OP.mult, op1=OP.add)
                nc.scalar.activation(out=sc[0:32, :], in_=wrap, func=AF.Sin)
                # cos(x) = sin((x - 2*pi*round((x + pi/2)/(2*pi))) + pi/2)
                wf2 = p_sm.tile([32, ts], F32, tag="wf")
                wi2 = p_sm.tile([32, ts], mybir.dt.int32, tag="wi")
                wrap2 = p_sm.tile([32, ts], F32, tag="wrap")
                nc.vector.tensor_scalar(out=wf2, in0=phr, scalar1=(1.0 / (2.0 * math.pi)),
                                        scalar2=0.25, op0=OP.mult, op1=OP.add)
                nc.vector.tensor_copy(out=wi2, in_=wf2)
                nc.vector.tensor_copy(out=wf2, in_=wi2)
                nc.vector.scalar_tensor_tensor(out=wrap2, in0=wf2, scalar=-TWO_PI,
                                               in1=phr, op0=OP.mult, op1=OP.add)
                nc.scalar.activation(out=sc[32:64, :], in_=wrap2, func=AF.Sin,
                                     bias=halfpi[0:32])
                # carry: c_phi = phr_last - 2*pi*round(phr_last/(2*pi))
                cwf = p_sm.tile([32, 1], F32, tag="cwf")
                cwi = p_sm.tile([32, 1], mybir.dt.int32, tag="cwi")
                nc.vector.tensor_scalar(out=cwf, in0=phr[:, ts - 1:ts],
                                        scalar1=(1.0 / (2.0 * math.pi)),
                                        scalar2=None, op0=OP.mult)
                nc.vector.tensor_copy(out=cwi, in_=cwf)
                nc.vector.tensor_copy(out=cwf, in_=cwi)
                nc.vector.scalar_tensor_tensor(out=c_phi, in0=cwf, scalar=-TWO_PI,
                                               in1=phr[:, ts - 1:ts],
                                               op0=OP.mult, op1=OP.add)
                wh = p_sm.tile([32, ts], F32, tag="wh")
                nc.scalar.activation(out=wh, in_=sms[32:64, :], func=AF.Tanh,
                                     scale=0.5, bias=dgb_col[32:64])
                w0 = p_sm.tile([32, ts], F16, tag="w0")
                nc.vector.tensor_scalar(out=w0, in0=wh, scalar1=0.5,
                                        scalar2=0.5, op0=OP.mult, op1=OP.add)
                if debug_taps:
                    nc.sync.dma_start(out=taps["d_w0"][:, t0:t0 + ts], in_=w0)
                    nc.sync.dma_start(out=taps["d_sc"][:, t0:t0 + ts], in_=sc)

                # ---- e assembly, integrator scan, Y, C ----
                for s in range(NSUP):
                    ytiles = []
                    for k in range(K):
                        sk = 2 * s + k
                        ephp = ps_eh.tile([128, ts], F32, tag="eph")
                        nc.tensor.matmul(
                            out=ephp,
                            lhsT=ct["c_eph"][:, 128 * (s * K + k):
                                             128 * (s * K + k + 1)],
                            rhs=sc, start=True, stop=True)
                        eh = p_eh.tile([128, ts + 1], F32, tag="eh")
                        nc.gpsimd.tensor_copy(out=eh[:, 0:1],
                                              in_=c_ep[:, sk:sk + 1])
                        nc.vector.scalar_tensor_tensor(
                            out=eh[:, 1:ts + 1], in0=ephp,
                            scalar=ct["c_berr"][:, k:k + 1],
                            in1=ez_t[(s, k)], op0=OP.add, op1=OP.add)
                        nc.gpsimd.tensor_copy(out=c_ep[:, sk:sk + 1],
                                              in_=eh[:, ts:ts + 1])
                        si = p_si.tile([128, ts], F32, tag="si")
                        nc.vector.tensor_tensor_scan(
                            out=si, data0=lam2_col.broadcast_to([128, ts]),
                            data1=eh[:, 1:ts + 1], initial=c_si[:, sk:sk + 1],
                            op0=OP.mult, op1=OP.add)
                        nc.gpsimd.tensor_copy(out=c_si[:, sk:sk + 1],
                                              in_=si[:, ts - 1:ts])
                        yk = p_yk.tile([128, ts], F32, tag="yk")
                        nc.vector.scalar_tensor_tensor(
                            out=yk, in0=si, scalar=co["ki_c1"],
                            in1=eh[:, 1:ts + 1], op0=OP.mult, op1=OP.add)
                        nc.vector.scalar_tensor_tensor(
                            out=yk, in0=eh[:, 0:ts], scalar=-co["kd_c1"],
                            in1=yk, op0=OP.mult, op1=OP.add)
                        if debug_taps and s == 0:
                            nc.sync.dma_start(out=taps["d_eh"][k][:, t0:t0 + ts],
                                              in_=eh[:, 1:ts + 1])
                            nc.sync.dma_start(out=taps["d_si"][k][:, t0:t0 + ts],
                                              in_=si)
                            nc.sync.dma_start(out=taps["d_yk"][k][:, t0:t0 + ts],
                                              in_=yk)
                        ytiles.append(yk)
                    dY = p_dy.tile([128, ts], F32, tag="dY")
                    nc.vector.tensor_tensor(out=dY, in0=ytiles[0],
                                            in1=ytiles[1], op=OP.subtract)
                    w0p = ps_w0.tile([128, ts], F32, tag="w0p")
                    nc.tensor.matmul(
                        out=w0p, lhsT=ct["c_w0b"][:, 128 * s:128 * (s + 1)],
                        rhs=w0, start=True, stop=True)
                    nc.vector.tensor_tensor(out=dY, in0=dY, in1=w0p, op=OP.mult)
                    nc.vector.tensor_tensor(
                        out=ca[:, s, R + t0: R + t0 + ts], in0=dY,
                        in1=ytiles[1], op=OP.add)
                    if debug_taps and s == 0:
                        nc.sync.dma_start(out=taps["d_C"][:, t0:t0 + ts],
                                          in_=ca[:, s, R + t0: R + t0 + ts])

        # ================= overlap-save sweep =================
        with ExitStack() as phbc:
            paw = phbc.enter_context(tc.tile_pool(name="paw", bufs=1))
            a_wide = paw.tile([128, NSUP, ncol], F32)
            with ExitStack() as phb:
                swp = phb.enter_context(tc.tile_pool(name="swp", bufs=3))
                ca4 = ca.rearrange("p g (c r) -> p g c r", r=R)
                aw4 = a_wide.rearrange("p g (c r) -> p g c r", r=R)
                nh = nch // 2

                def tsl(t4, j, grp):
                    # chunk-group slices (even/odd) to run two independent
                    # dependency chains and hide per-step latency
                    if j < R:
                        return t4[:, :, grp:nch:2, j]
                    return t4[:, :, 1 + grp:nch + 1:2, j - R]

                for grp in range(2):
                    nc.vector.memset(tsl(aw4, W - 1, grp), 0.0)
                # D-state pre-scaled by kappa so u = C - Dk is a plain
                # subtract (Pool-eligible); Dk' = lam2*Dk + kappa*rate*r
                #                                  - kappa*beta*q
                kr = co["kappa"] * co["rate"]
                kb = co["kappa"] * co["beta"]
                d_prev = [None, None]
                for i in range(R + W):
                    for grp in range(2):
                        ci = tsl(ca4, i + W, grp)
                        a_prev = tsl(aw4, i + W - 1, grp)
                        h = swp.tile([128, NSUP, nh], F32, tag=f"h{grp}")
                        if d_prev[grp] is None:
                            nc.scalar.activation(out=h, in_=ci, func=AF.Tanh)
                        else:
                            u = swp.tile([128, NSUP, nh], F32, tag=f"u{grp}")
                            nc.gpsimd.tensor_tensor(out=u, in0=ci,
                                                    in1=d_prev[grp],
                                                    op=OP.subtract)
                            nc.scalar.activation(out=h, in_=u, func=AF.Tanh)
                        q = swp.tile([128, NSUP, nh], F32, tag=f"q{grp}")
                        nc.gpsimd.tensor_tensor(out=q, in0=h, in1=a_prev,
                                                op=OP.subtract)
                        r = swp.tile([128, NSUP, nh], F32, tag=f"r{grp}")
                        nc.scalar.activation(out=r, in_=q, func=AF.Tanh,
                                             scale=co["s2"])
                        m = swp.tile([128, NSUP, nh], F32, tag=f"m{grp}")
                        nc.vector.tensor_scalar(out=m, in0=r, scalar1=co["rate"],
                                                scalar2=None, op0=OP.mult)
                        mk = swp.tile([128, NSUP, nh], F32, tag=f"mk{grp}")
                        nc.gpsimd.tensor_scalar(out=mk, in0=r, scalar1=kr,
                                                scalar2=None, op0=OP.mult)
                        nc.gpsimd.tensor_tensor(out=tsl(aw4, i + W, grp),
                                                 in0=a_prev, in1=m, op=OP.add)
                        dtmp = swp.tile([128, NSUP, nh], F32, tag=f"dt{grp}")
                        nc.vector.scalar_tensor_tensor(
                            out=dtmp, in0=q, scalar=-kb, in1=mk,
                            op0=OP.mult, op1=OP.add)
                        d_new = swp.tile([128, NSUP, nh], F32, tag=f"dn{grp}")
                        if d_prev[grp] is None:
                            nc.vector.tensor_scalar(out=d_new, in0=dtmp,
                                                    scalar1=1.0, scalar2=None,
                                                    op0=OP.mult)
                        else:
                            nc.vector.scalar_tensor_tensor(
                                out=d_new, in0=d_prev[grp], scalar=co["lam2"],
                                in1=dtmp, op0=OP.mult, op1=OP.add)
                        d_prev[grp] = d_new

            # ============= output transpose + store =============
            with ExitStack() as phc:
                p_o = phc.enter_context(tc.tile_pool(name="po", bufs=3))
                ps_o = phc.enter_context(tc.tile_pool(name="pso", bufs=2,
                                                      space="PSUM"))
                for tau in range(t_total // 128):
                    ot = p_o.tile([128, NSUP * 128], F32, tag="ot")
                    for g in range(NSUP):
                        tp = ps_o.tile([128, 128], F32, tag="otp")
                        nc.tensor.transpose(
                            tp, a_wide[:, g, R + 128 * tau: R + 128 * (tau + 1)],
                            ident)
                        nc.scalar.copy(out=ot[:, 128 * g: 128 * (g + 1)], in_=tp)
                    nc.sync.dma_start(
                        out=out_d[128 * tau: 128 * (tau + 1)]
                        .rearrange("t b a -> t (b a)"), in_=ot)
    return nc


def _in_maps(inputs, consts):
    x = np.ascontiguousarray(np.asarray(inputs["states"], np.float32))
    maps = []
    for j in range(NCORES):
        m = {"x": np.ascontiguousarray(x[:, BL * j: BL * (j + 1), :])}
        m.update(consts)
        maps.append(m)
    return maps


def kernel(**inputs):
    co, consts = _coeffs(inputs)
    nc = bacc.Bacc("TRN2", num_devices=NCORES)
    build_program(nc, co)
    nc.compile()
    maps = _in_maps(inputs, consts)
    res = run_bass_kernel_spmd(nc, maps, list(range(NCORES)))
    outs = [np.asarray(res.results[j]["out"]).reshape(T_FULL, BL, A)
            for j in range(NCORES)]
    return np.concatenate(outs, axis=1)



# revision 3
# speedup vs baseline: 18.1802x; 18.1802x over previous
"""Trainium2 Bass kernel for nn_ControlPolicy (T=4096, B=256, N=64, K=2, A=16).

Sharding: data-parallel over the batch axis B across 8 NeuronCores (32 rows
per core); tiny parameters replicated.

Per-core algorithm:
  All linear recurrences (z low-pass filter, phase integrator, i_s
  integrator) run at line rate on the DVE tensor_tensor_scan instruction with
  time on the free axis.  With sum_k softmax(w)_k == 1 and uniform kp/ki/kd
  (np.full in setup_inputs), the PID algebra collapses so the only truly
  sequential computation is a small nonlinear recurrence in (a, D):

      u = C_t - kappa*D ; h = tanh(u) ; q = h - a ; r = tanh(s2*q)
      a' = a + rate*r   ; D' = lam2*D + rate*r - beta*q

  where C_t is bulk-precomputable.  This is evaluated with an overlap-save
  chunked sweep: T is cut into chunks of R=64 steps, every chunk starts from
  zero state W=32 steps early, and the contraction of the per-step map washes
  out the wrong start (validated to ~1e-5 abs against the jax reference).
  All 64 chunks advance simultaneously inside each DVE/ACT instruction.

Streaming phase per superblock of 512 t:
  layernorm (reduce stats + broadcast apply) -> PE transpose to [(b,n),t] ->
  PE projections (e-features per k, omega/gate smalls) -> alpha-filter scans
  -> smalls pipeline (omega, phi scan+wrap, sin/cos, gate sigmoid) ->
  phase-feature matmuls -> e assembly -> integrator scan -> C assembly.
The C and a time-buffers share one SBUF allocation: the sweep's a-writes land
exactly on C columns already consumed in the same step.
"""
import math
import numpy as np
from contextlib import ExitStack

import concourse.bass as bass
import concourse.bacc as bacc
import concourse.tile as tile
from concourse import mybir
from concourse.bass_utils import run_bass_kernel_spmd
from concourse.masks import make_identity

F32 = mybir.dt.float32
F16 = mybir.dt.float16
OP = mybir.AluOpType
AF = mybir.ActivationFunctionType
AX = mybir.AxisListType

T_FULL = 4096
B_FULL = 256
N = 64
K = 2
A = 16
NCORES = 8
BL = B_FULL // NCORES          # 32
LN_EPS = 1e-5
TWO_PI = float(np.float32(2.0 * np.pi))

R = 64                          # sweep chunk length
W = 32                          # sweep warm-up
NSUP = 4                        # supersets of 8 b-rows
NPAIR = BL // 2                 # 16


def _sigmoid(x): return 1.0 / (1.0 + math.exp(-x))
def _softplus(x): return math.log1p(math.exp(x))


def _coeffs(inputs):
    f = lambda k: float(np.asarray(inputs[k], np.float64))
    alpha = _sigmoid(f("filter_alpha_logit"))
    leak = _sigmoid(f("int_leak_logit"))
    beta = _sigmoid(f("act_beta_logit"))
    rate = 0.25 * _sigmoid(f("rate_limit_raw"))
    aw = _softplus(f("aw_gain_raw"))
    omega_base = _softplus(f("phase_omega_raw")) + 0.001

    kp_a = np.log1p(np.exp(np.asarray(inputs["kp_raw"], np.float64)))
    ki_a = np.log1p(np.exp(np.asarray(inputs["ki_raw"], np.float64)))
    kd_a = np.log1p(np.exp(np.asarray(inputs["kd_raw"], np.float64)))
    for nm, arr in (("kp", kp_a), ("ki", ki_a), ("kd", kd_a)):
        assert np.allclose(arr, arr.flat[0], rtol=1e-12), f"{nm} not uniform"
    kp, ki, kd = float(kp_a.flat[0]), float(ki_a.flat[0]), float(kd_a.flat[0])

    lam2 = 1.0 - leak
    c1 = kp + kd
    co = dict(
        alpha=alpha, lam=1.0 - alpha, lam2=lam2, beta=beta, rate=rate,
        omega_base=omega_base, c1=c1, kappa=ki * aw * lam2,
        s2=beta / (rate + 1e-6), ki_c1=ki / c1, kd_c1=kd / c1,
    )

    ln_w = np.asarray(inputs["ln_w"], np.float64)
    ln_b = np.asarray(inputs["ln_b"], np.float64)
    w_state = np.asarray(inputs["w_state"], np.float64)
    w_phase = np.asarray(inputs["w_phase"], np.float64)
    b_err = np.asarray(inputs["b_err"], np.float64)
    gate_w = np.asarray(inputs["gate_w"], np.float64)
    gate_b = np.asarray(inputs["gate_b"], np.float64)
    pos = np.asarray(inputs["phase_omega_state"], np.float64)

    # e-feature projection lhsT [128, K*4*128]: accumulation-chain blocks.
    # Block (k, pw) maps pair pw's partitions (bw2, n64) onto out rows
    # (b8, a16) = 16*(2*pw+bw) + a of the k psum tile.
    we = np.zeros((128, K * 4 * 128), np.float64)
    for k in range(K):
        for pw in range(4):
            base = (k * 4 + pw) * 128
            for bw in range(2):
                for a in range(A):
                    we[bw * 64:(bw + 1) * 64,
                       base + 16 * (2 * pw + bw) + a] = (
                        c1 * alpha * w_state[k, a, :] * ln_w)
    # smalls projection lhsT [128, 16*64]: accumulation-chain blocks per pair.
    # Out rows type-major: 32*c + b with c in {pos, gdiff = g0 - g1}.
    wsm_rows = [alpha * pos * ln_w, alpha * (gate_w[0] - gate_w[1]) * ln_w]
    wsm = np.zeros((128, 16 * 64), np.float64)
    for p in range(16):
        for bw in range(2):
            for c in range(2):
                wsm[bw * 64:(bw + 1) * 64,
                    p * 64 + 32 * c + 2 * p + bw] = wsm_rows[c]
    # projection biases from ln_b (zero in practice -> matmuls skipped)
    be = np.stack([c1 * alpha * (w_state[k] @ ln_b) for k in range(K)])
    bsm = np.array([alpha * (pos @ ln_b),
                    alpha * ((gate_w[0] - gate_w[1]) @ ln_b)])
    co["has_proj_bias"] = bool(np.any(be != 0) or np.any(bsm != 0))
    ebias = np.zeros((1, 128 * K), np.float64)
    for k in range(K):
        ebias[0, 128 * k:128 * (k + 1)] = np.tile(be[k], 8)
    smbias = np.repeat(bsm, 32).reshape(1, 64)

    eph = np.zeros((64, NSUP * K * 128), np.float64)
    for s in range(NSUP):
        for k in range(K):
            base = (s * K + k) * 128
            for bb in range(8):
                for a in range(A):
                    eph[8 * s + bb, base + 16 * bb + a] = c1 * w_phase[k, a, 0]
                    eph[32 + 8 * s + bb, base + 16 * bb + a] = c1 * w_phase[k, a, 1]
    w0b = np.zeros((32, NSUP * 128), np.float64)
    for s in range(NSUP):
        for bb in range(8):
            for a in range(A):
                w0b[8 * s + bb, s * 128 + 16 * bb + a] = 1.0
    berr = np.zeros((128, K), np.float64)
    for k in range(K):
        berr[:, k] = np.tile(c1 * b_err[k], 8)

    co["dgb"] = float(gate_b[0] - gate_b[1])
    consts = {nm: arr.astype(np.float16) for nm, arr in dict(
        c_we=we, c_wsm=wsm, c_eph=eph, c_w0b=w0b).items()}
    consts.update({nm: arr.astype(np.float32) for nm, arr in dict(
        c_berr=berr, c_ebias=ebias, c_smbias=smbias).items()})
    return co, consts


def build_program(nc, co, t_total=T_FULL, ts=512, debug_taps=False, reps=1):
    nsb = t_total // ts
    nch = t_total // R
    ncol = R * (nch + 1)
    ntc = ts // 128                 # 128-row t-chunks per superblock

    x_in = nc.dram_tensor("x", [t_total, BL, N], F32, kind="ExternalInput").ap()
    out_d = nc.dram_tensor("out", [t_total, BL, A], F32, kind="ExternalOutput").ap()
    shapes = dict(c_we=(128, K * 4 * 128), c_wsm=(128, 16 * 64),
                  c_eph=(64, NSUP * K * 128), c_w0b=(32, NSUP * 128),
                  c_berr=(128, K), c_ebias=(1, 128 * K), c_smbias=(1, 64))
    half = {"c_we", "c_wsm", "c_eph", "c_w0b"}
    cw = {nm: nc.dram_tensor(nm, list(sh), F16 if nm in half else F32,
                             kind="ExternalInput").ap()
          for nm, sh in shapes.items()}
    taps = {}
    if debug_taps:
        for spec in [("d_xn", (128, BL * N), F16), ("d_ez", (128, t_total)),
                     ("d_eh", (K, 128, t_total)), ("d_si", (K, 128, t_total)),
                     ("d_yk", (K, 128, t_total)), ("d_w0", (32, t_total), F16),
                     ("d_sc", (64, t_total), F16), ("d_C", (128, t_total))]:
            nm, sh = spec[0], spec[1]
            dt_ = spec[2] if len(spec) > 2 else F32
            taps[nm] = nc.dram_tensor(nm, list(sh), dt_, kind="ExternalOutput").ap()
    for _rep in range(reps):
        _build_body(nc, co, x_in, out_d, cw, taps, t_total, ts, debug_taps,
                    nsb, nch, ncol, ntc)
    return nc


def _build_body(nc, co, x_in, out_d, cw, taps, t_total, ts, debug_taps, nsb,
                nch, ncol, ntc):

    with tile.TileContext(nc) as tc, ExitStack() as top:
        consts = top.enter_context(tc.tile_pool(name="consts", bufs=1))
        carry = top.enter_context(tc.tile_pool(name="carry", bufs=1))
        bigp = top.enter_context(tc.tile_pool(name="big", bufs=1))

        ct = {}
        for nm, ap in cw.items():
            t = consts.tile(list(ap.shape), ap.dtype, tag=nm)
            nc.sync.dma_start(out=t, in_=ap)
            ct[nm] = t
        ident = consts.tile([128, 128], F32)
        make_identity(nc, ident)
        ident_h = consts.tile([128, 128], F16)
        nc.vector.tensor_copy(out=ident_h, in_=ident)
        eps_col = consts.tile([128, 1], F32); nc.vector.memset(eps_col, LN_EPS)
        halfpi = consts.tile([128, 1], F32); nc.vector.memset(halfpi, math.pi / 2)
        lam_col = consts.tile([128, 1], F32); nc.vector.memset(lam_col, co["lam"])
        lam2_col = consts.tile([128, 1], F32); nc.vector.memset(lam2_col, co["lam2"])
        one_col = consts.tile([128, 1], F32); nc.vector.memset(one_col, 1.0)
        dgb_col = consts.tile([64, 1], F32); nc.vector.memset(dgb_col, 0.5 * co["dgb"])
        ones_row = consts.tile([1, ts], F32); nc.vector.memset(ones_row, 1.0)

        c_ez = carry.tile([128, 2 * NSUP], F32); nc.vector.memset(c_ez, 0.0)
        c_si = carry.tile([128, 2 * NSUP], F32); nc.vector.memset(c_si, 0.0)
        c_ep = carry.tile([128, 2 * NSUP], F32); nc.vector.memset(c_ep, 0.0)
        c_sm = carry.tile([64, 1], F32); nc.vector.memset(c_sm, 0.0)
        c_phi = carry.tile([32, 1], F32); nc.vector.memset(c_phi, 0.0)

        # C time buffer (head zero-padded for chunk-0 warm-up), fp16
        ca = bigp.tile([128, NSUP, ncol], F16)
        for g in range(NSUP):
            nc.vector.memset(ca[:, g, 0:R], 0.0)

        # ================= streaming phase =================
        with ExitStack() as pha:
            p_x = pha.enter_context(tc.tile_pool(name="px", bufs=4))
            p_xsq = pha.enter_context(tc.tile_pool(name="pxsq", bufs=2))
            p_xh = pha.enter_context(tc.tile_pool(name="pxh", bufs=4))
            p_st = pha.enter_context(tc.tile_pool(name="pst", bufs=1))
            p_xnt = pha.enter_context(tc.tile_pool(name="pxnt", bufs=6))
            p_ez = pha.enter_context(tc.tile_pool(name="pez", bufs=9))
            p_eh = pha.enter_context(tc.tile_pool(name="peh", bufs=3))
            p_si = pha.enter_context(tc.tile_pool(name="psi", bufs=2))
            p_yk = pha.enter_context(tc.tile_pool(name="pyk", bufs=3))
            p_dy = pha.enter_context(tc.tile_pool(name="pdy", bufs=2))
            p_sm = pha.enter_context(tc.tile_pool(name="psm", bufs=1))
            ps_tp = pha.enter_context(tc.tile_pool(name="pstp", bufs=2, space="PSUM"))
            ps_ep = pha.enter_context(tc.tile_pool(name="psep", bufs=2, space="PSUM"))
            ps_eh = pha.enter_context(tc.tile_pool(name="pseh", bufs=2, space="PSUM"))
            ps_w0 = pha.enter_context(tc.tile_pool(name="psw0", bufs=1, space="PSUM"))
            ps_sm = pha.enter_context(tc.tile_pool(name="pssm", bufs=1, space="PSUM"))

            for sb in range(nsb):
                t0 = sb * ts
                # ---- load + layernorm (layout [t, (b,n)]) ----
                xch = []
                for c in range(ntc):
                    xt = p_x.tile([128, BL * N], F32, tag="xch")
                    nc.sync.dma_start(
                        out=xt,
                        in_=x_in[t0 + 128 * c: t0 + 128 * (c + 1)]
                        .rearrange("t b n -> t (b n)"))
                    xch.append(xt)
                # stats for all chunks batched into [128, ntc, BL] tiles
                s1c = p_st.tile([128, ntc, BL], F32, tag="s1c")
                s2c = p_st.tile([128, ntc, BL], F32, tag="s2c")
                for c in range(ntc):
                    xt = xch[c]
                    x3 = xt.rearrange("t (b n) -> t b n", b=BL)
                    xsq = p_xsq.tile([128, BL * N], F32, tag="xsq")
                    nc.scalar.activation(out=xsq, in_=xt, func=AF.Square)
                    nc.vector.tensor_reduce(out=s1c[:, c, :], in_=x3, axis=AX.X,
                                            op=OP.add)
                    nc.vector.tensor_reduce(
                        out=s2c[:, c, :],
                        in_=xsq.rearrange("t (b n) -> t b n", b=BL),
                        axis=AX.X, op=OP.add)
                # mu, var+eps, invstd = fast-rsqrt + 2 Newton steps (all DVE)
                mu = p_st.tile([128, ntc, BL], F32, tag="mu")
                nc.vector.tensor_scalar(out=mu, in0=s1c, scalar1=1.0 / N,
                                        scalar2=None, op0=OP.mult)
                msq = p_st.tile([128, ntc, BL], F32, tag="msq")
                nc.vector.tensor_tensor(out=msq, in0=mu, in1=mu, op=OP.mult)
                vpe = s2c
                nc.vector.scalar_tensor_tensor(
                    out=vpe, in0=s2c, scalar=1.0 / N, in1=msq,
                    op0=OP.mult, op1=OP.subtract)
                nc.vector.tensor_scalar(out=vpe, in0=vpe, scalar1=LN_EPS,
                                        scalar2=None, op0=OP.add)
                vh = msq
                nc.vector.tensor_scalar(out=vh, in0=vpe, scalar1=0.5,
                                        scalar2=None, op0=OP.mult)
                inv = p_st.tile([128, ntc, BL], F32, tag="inv")
                ivi = inv.bitcast(mybir.dt.int32)
                nc.vector.tensor_scalar(out=ivi, in0=vpe.bitcast(mybir.dt.int32),
                                        scalar1=1, scalar2=None,
                                        op0=OP.arith_shift_right)
                nc.vector.tensor_scalar(out=ivi, in0=ivi, scalar1=-1,
                                        scalar2=0x5f3759df, op0=OP.mult,
                                        op1=OP.add)
                for _ in range(2):
                    yy = p_st.tile([128, ntc, BL], F32, tag="yy")
                    nc.vector.tensor_tensor(out=yy, in0=inv, in1=inv, op=OP.mult)
                    nc.vector.tensor_tensor(out=yy, in0=yy, in1=vh, op=OP.mult)
                    nc.vector.tensor_scalar(out=yy, in0=yy, scalar1=-1.0,
                                            scalar2=1.5, op0=OP.mult, op1=OP.add)
                    nc.vector.tensor_tensor(out=inv, in0=inv, in1=yy, op=OP.mult)
                xnh = []
                for c in range(ntc):
                    xt = xch[c]
                    x3 = xt.rearrange("t (b n) -> t b n", b=BL)
                    nc.gpsimd.tensor_tensor(
                        out=x3, in0=x3,
                        in1=mu[:, c, :, None].broadcast_to([128, BL, N]),
                        op=OP.subtract)
                    xh = p_xh.tile([128, BL * N], F16, tag="xnh")
                    nc.gpsimd.tensor_tensor(
                        out=xh.rearrange("t (b n) -> t b n", b=BL), in0=x3,
                        in1=inv[:, c, :, None].broadcast_to([128, BL, N]),
                        op=OP.mult)
                    xnh.append(xh)
                    if debug_taps and sb == 0 and c == 0:
                        nc.sync.dma_start(out=taps["d_xn"], in_=xh)

                # ---- transpose + projections + filter scans ----
                smp = ps_sm.tile([64, ts], F32, tag="smp")
                ez_t = {}
                for s in range(NSUP):
                    eps_ = {}
                    for k in range(K):
                        epk = ps_ep.tile([128, ts], F32, tag="epre")
                        eps_[k] = epk
                    for pw in range(4):
                        p = 4 * s + pw
                        tpt = ps_tp.tile([128, ts], F16, tag="tp")
                        for c in range(ntc):
                            nc.tensor.transpose(
                                tpt[:, 128 * c: 128 * (c + 1)],
                                xnh[c][:, 128 * p: 128 * (p + 1)], ident_h)
                        xT = p_xnt.tile([128, ts], F16, tag="xnT")
                        nc.scalar.copy(out=xT, in_=tpt)
                        for k in range(K):
                            nc.tensor.matmul(
                                out=eps_[k],
                                lhsT=ct["c_we"][:, (k * 4 + pw) * 128:
                                                (k * 4 + pw + 1) * 128],
                                rhs=xT, start=(pw == 0), stop=(pw == 3))
                        nc.tensor.matmul(out=smp,
                                         lhsT=ct["c_wsm"][:, p * 64:(p + 1) * 64],
                                         rhs=xT, start=(p == 0), stop=(p == 15))
                    for k in range(K):
                        if co["has_proj_bias"]:
                            nc.tensor.matmul(
                                out=eps_[k],
                                lhsT=ct["c_ebias"][:, 128 * k:128 * (k + 1)],
                                rhs=ones_row, start=False, stop=True,
                                skip_group_check=True)
                        ez = p_ez.tile([128, ts], F32, tag="ez")
                        sk = 2 * s + k
                        nc.vector.tensor_tensor_scan(
                            out=ez, data0=lam_col.broadcast_to([128, ts]),
                            data1=eps_[k], initial=c_ez[:, sk:sk + 1],
                            op0=OP.mult, op1=OP.add)
                        nc.gpsimd.tensor_copy(out=c_ez[:, sk:sk + 1],
                                              in_=ez[:, ts - 1:ts])
                        if debug_taps and s == 0 and k == 0:
                            nc.sync.dma_start(out=taps["d_ez"][:, t0:t0 + ts],
                                              in_=ez)
                        ez_t[(s, k)] = ez
                if co["has_proj_bias"]:
                    nc.tensor.matmul(out=smp, lhsT=ct["c_smbias"], rhs=ones_row,
                                     start=False, stop=True,
                                     skip_group_check=True)

                # ---- smalls pipeline ----
                sms = p_sm.tile([64, ts], F32, tag="sms")
                nc.vector.tensor_tensor_scan(
                    out=sms, data0=lam_col[0:64].broadcast_to([64, ts]),
                    data1=smp, initial=c_sm, op0=OP.mult, op1=OP.add)
                nc.gpsimd.tensor_copy(out=c_sm, in_=sms[:, ts - 1:ts])
                sigp = p_sm.tile([32, ts], F32, tag="sigp")
                nc.scalar.activation(out=sigp, in_=sms[0:32, :], func=AF.Tanh)
                om = p_sm.tile([32, ts], F32, tag="om")
                nc.vector.tensor_scalar(out=om, in0=sigp, scalar1=0.02,
                                        scalar2=co["omega_base"],
                                        op0=OP.mult, op1=OP.add)
                nc.vector.tensor_scalar(out=om, in0=om, scalar1=1.0,
                                        scalar2=0.001, op0=OP.min, op1=OP.max)
                phr = p_sm.tile([32, ts], F32, tag="phr")
                nc.vector.tensor_tensor_scan(
                    out=phr, data0=one_col[0:32].broadcast_to([32, ts]),
                    data1=om, initial=c_phi, op0=OP.mult, op1=OP.add)
                # wrap to [-pi, pi] via round-to-nearest int convert:
                #   w = x - 2*pi*round(x/(2*pi)); sin periodic, so sin(w)=sin(x)
                sc = p_sm.tile([64, ts], F16, tag="sc")
                wf = p_sm.tile([32, ts], F32, tag="wf")
                wi = p_sm.tile([32, ts], mybir.dt.int32, tag="wi")
                wrap = p_sm.tile([32, ts], F32, tag="wrap")
                nc.vector.tensor_scalar(out=wf, in0=phr, scalar1=(1.0 / (2.0 * math.pi)),
                                        scalar2=None, op0=OP.mult)
                nc.vector.tensor_copy(out=wi, in_=wf)
                nc.vector.tensor_copy(out=wf, in_=wi)
                nc.vector.scalar_tensor_tensor(out=wrap, in0=wf, scalar=-TWO_PI,
                                               in1=phr, op0=OP.mult, op1=OP.add)
                nc.scalar.activation(out=sc[0:32, :], in_=wrap, func=AF.Sin)
                # cos(x) = sin((x - 2*pi*round((x + pi/2)/(2*pi))) + pi/2)
                wf2 = p_sm.tile([32, ts], F32, tag="wf")
                wi2 = p_sm.tile([32, ts], mybir.dt.int32, tag="wi")
                wrap2 = p_sm.tile([32, ts], F32, tag="wrap")
                nc.vector.tensor_scalar(out=wf2, in0=phr, scalar1=(1.0 / (2.0 * math.pi)),
                                        scalar2=0.25, op0=OP.mult, op1=OP.add)
                nc.vector.tensor_copy(out=wi2, in_=wf2)
                nc.vector.tensor_copy(out=wf2, in_=wi2)
                nc.vector.scalar_tensor_tensor(out=wrap2, in0=wf2, scalar=-TWO_PI,
                                               in1=phr, op0=OP.mult, op1=OP.add)
                nc.scalar.activation(out=sc[32:64, :], in_=wrap2, func=AF.Sin,
                                     bias=halfpi[0:32])
                # carry: c_phi = phr_last - 2*pi*round(phr_last/(2*pi))
                cwf = p_sm.tile([32, 1], F32, tag="cwf")
                cwi = p_sm.tile([32, 1], mybir.dt.int32, tag="cwi")
                nc.vector.tensor_scalar(out=cwf, in0=phr[:, ts - 1:ts],
                                        scalar1=(1.0 / (2.0 * math.pi)),
                                        scalar2=None, op0=OP.mult)
                nc.vector.tensor_copy(out=cwi, in_=cwf)
                nc.vector.tensor_copy(out=cwf, in_=cwi)
                nc.vector.scalar_tensor_tensor(out=c_phi, in0=cwf, scalar=-TWO_PI,
                                               in1=phr[:, ts - 1:ts],
                                               op0=OP.mult, op1=OP.add)
                wh = p_sm.tile([32, ts], F32, tag="wh")
                nc.scalar.activation(out=wh, in_=sms[32:64, :], func=AF.Tanh,
                                     scale=0.5, bias=dgb_col[32:64])
                w0 = p_sm.tile([32, ts], F16, tag="w0")
                nc.vector.tensor_scalar(out=w0, in0=wh, scalar1=0.5,
                                        scalar2=0.5, op0=OP.mult, op1=OP.add)
                if debug_taps:
                    nc.sync.dma_start(out=taps["d_w0"][:, t0:t0 + ts], in_=w0)
                    nc.sync.dma_start(out=taps["d_sc"][:, t0:t0 + ts], in_=sc)

                # ---- e assembly, integrator scan, Y, C ----
                for s in range(NSUP):
                    ytiles = []
                    for k in range(K):
                        sk = 2 * s + k
                        ephp = ps_eh.tile([128, ts], F32, tag="eph")
                        nc.tensor.matmul(
                            out=ephp,
                            lhsT=ct["c_eph"][:, 128 * (s * K + k):
                                             128 * (s * K + k + 1)],
                            rhs=sc, start=True, stop=True)
                        eh = p_eh.tile([128, ts + 1], F32, tag="eh")
                        nc.gpsimd.tensor_copy(out=eh[:, 0:1],
                                              in_=c_ep[:, sk:sk + 1])
                        nc.vector.scalar_tensor_tensor(
                            out=eh[:, 1:ts + 1], in0=ephp,
                            scalar=ct["c_berr"][:, k:k + 1],
                            in1=ez_t[(s, k)], op0=OP.add, op1=OP.add)
                        nc.gpsimd.tensor_copy(out=c_ep[:, sk:sk + 1],
                                              in_=eh[:, ts:ts + 1])
                        si = p_si.tile([128, ts], F32, tag="si")
                        nc.vector.tensor_tensor_scan(
                            out=si, data0=lam2_col.broadcast_to([128, ts]),
                            data1=eh[:, 1:ts + 1], initial=c_si[:, sk:sk + 1],
                            op0=OP.mult, op1=OP.add)
                        nc.gpsimd.tensor_copy(out=c_si[:, sk:sk + 1],
                                              in_=si[:, ts - 1:ts])
                        yk = p_yk.tile([128, ts], F32, tag="yk")
                        nc.vector.scalar_tensor_tensor(
                            out=yk, in0=si, scalar=co["ki_c1"],
                            in1=eh[:, 1:ts + 1], op0=OP.mult, op1=OP.add)
                        nc.vector.scalar_tensor_tensor(
                            out=yk, in0=eh[:, 0:ts], scalar=-co["kd_c1"],
                            in1=yk, op0=OP.mult, op1=OP.add)
                        if debug_taps and s == 0:
                            nc.sync.dma_start(out=taps["d_eh"][k][:, t0:t0 + ts],
                                              in_=eh[:, 1:ts + 1])
                            nc.sync.dma_start(out=taps["d_si"][k][:, t0:t0 + ts],
                                              in_=si)
                            nc.sync.dma_start(out=taps["d_yk"][k][:, t0:t0 + ts],
                                              in_=yk)
                        ytiles.append(yk)
                    dY = p_dy.tile([128, ts], F32, tag="dY")
                    nc.vector.tensor_tensor(out=dY, in0=ytiles[0],
                                            in1=ytiles[1], op=OP.subtract)
                    w0p = ps_w0.tile([128, ts], F32, tag="w0p")
                    nc.tensor.matmul(
                        out=w0p, lhsT=ct["c_w0b"][:, 128 * s:128 * (s + 1)],
                        rhs=w0, start=True, stop=True)
                    nc.vector.tensor_tensor(out=dY, in0=dY, in1=w0p, op=OP.mult)
                    nc.vector.tensor_tensor(
                        out=ca[:, s, R + t0: R + t0 + ts], in0=dY,
                        in1=ytiles[1], op=OP.add)
                    if debug_taps and s == 0:
                        nc.sync.dma_start(out=taps["d_C"][:, t0:t0 + ts],
                                          in_=ca[:, s, R + t0: R + t0 + ts])

        # ================= overlap-save sweep =================
        with ExitStack() as phbc:
            paw = phbc.enter_context(tc.tile_pool(name="paw", bufs=1))
            a_wide = paw.tile([128, NSUP, ncol], F32)
            with ExitStack() as phb:
                swp = phb.enter_context(tc.tile_pool(name="swp", bufs=3))
                ca4 = ca.rearrange("p g (c r) -> p g c r", r=R)
                aw4 = a_wide.rearrange("p g (c r) -> p g c r", r=R)
                nh = nch // 2

                def tsl(t4, j, grp):
                    # chunk-group slices (even/odd) to run two independent
                    # dependency chains and hide per-step latency
                    if j < R:
                        return t4[:, :, grp:nch:2, j]
                    return t4[:, :, 1 + grp:nch + 1:2, j - R]

                for grp in range(2):
                    nc.vector.memset(tsl(aw4, W - 1, grp), 0.0)
                # D-state pre-scaled by kappa so u = C - Dk is a plain
                # subtract (Pool-eligible); Dk' = lam2*Dk + kappa*rate*r
                #                                  - kappa*beta*q
                kr = co["kappa"] * co["rate"]
                kb = co["kappa"] * co["beta"]
                d_prev = [None, None]
                for i in range(R + W):
                    for grp in range(2):
                        ci = tsl(ca4, i + W, grp)
                        a_prev = tsl(aw4, i + W - 1, grp)
                        h = swp.tile([128, NSUP, nh], F32, tag=f"h{grp}")
                        if d_prev[grp] is None:
                            nc.scalar.activation(out=h, in_=ci, func=AF.Tanh)
                        else:
                            u = swp.tile([128, NSUP, nh], F32, tag=f"u{grp}")
                            nc.gpsimd.tensor_tensor(out=u, in0=ci,
                                                    in1=d_prev[grp],
                                                    op=OP.subtract)
                            nc.scalar.activation(out=h, in_=u, func=AF.Tanh)
                        q = swp.tile([128, NSUP, nh], F32, tag=f"q{grp}")
                        nc.gpsimd.tensor_tensor(out=q, in0=h, in1=a_prev,
                                                op=OP.subtract)
                        r = swp.tile([128, NSUP, nh], F32, tag=f"r{grp}")
                        nc.scalar.activation(out=r, in_=q, func=AF.Tanh,
                                             scale=co["s2"])
                        m = swp.tile([128, NSUP, nh], F32, tag=f"m{grp}")
                        nc.vector.tensor_scalar(out=m, in0=r, scalar1=co["rate"],
                                                scalar2=None, op0=OP.mult)
                        mk = swp.tile([128, NSUP, nh], F32, tag=f"mk{grp}")
                        nc.gpsimd.tensor_scalar(out=mk, in0=r, scalar1=kr,
                                                scalar2=None, op0=OP.mult)
                        nc.gpsimd.tensor_tensor(out=tsl(aw4, i + W, grp),
                                                 in0=a_prev, in1=m, op=OP.add)
                        dtmp = swp.tile([128, NSUP, nh], F32, tag=f"dt{grp}")
                        nc.vector.scalar_tensor_tensor(
                            out=dtmp, in0=q, scalar=-kb, in1=mk,
                            op0=OP.mult, op1=OP.add)
                        d_new = swp.tile([128, NSUP, nh], F32, tag=f"dn{grp}")
                        if d_prev[grp] is None:
                            nc.vector.tensor_scalar(out=d_new, in0=dtmp,
                                                    scalar1=1.0, scalar2=None,
                                                    op0=OP.mult)
                        else:
                            nc.vector.scalar_tensor_tensor(
                                out=d_new, in0=d_prev[grp], scalar=co["lam2"],
                                in1=dtmp, op0=OP.mult, op1=OP.add)
                        d_prev[grp] = d_new

            # ============= output transpose + store =============
            with ExitStack() as phc:
                p_o = phc.enter_context(tc.tile_pool(name="po", bufs=3))
                ps_o = phc.enter_context(tc.tile_pool(name="pso", bufs=2,
                                                      space="PSUM"))
                for tau in range(t_total // 128):
                    ot = p_o.tile([128, NSUP * 128], F32, tag="ot")
                    for g in range(NSUP):
                        tp = ps_o.tile([128, 128], F32, tag="otp")
                        nc.tensor.transpose(
                            tp, a_wide[:, g, R + 128 * tau: R + 128 * (tau + 1)],
                            ident)
                        nc.scalar.copy(out=ot[:, 128 * g: 128 * (g + 1)], in_=tp)
                    nc.sync.dma_start(
                        out=out_d[128 * tau: 128 * (tau + 1)]
                        .rearrange("t b a -> t (b a)"), in_=ot)
    return nc


def _in_maps(inputs, consts):
    x = np.ascontiguousarray(np.asarray(inputs["states"], np.float32))
    maps = []
    for j in range(NCORES):
        m = {"x": np.ascontiguousarray(x[:, BL * j: BL * (j + 1), :])}
        m.update(consts)
        maps.append(m)
    return maps


def kernel(**inputs):
    co, consts = _coeffs(inputs)
    nc = bacc.Bacc("TRN2", num_devices=NCORES)
    build_program(nc, co)
    nc.compile()
    maps = _in_maps(inputs, consts)
    res = run_bass_kernel_spmd(nc, maps, list(range(NCORES)))
    outs = [np.asarray(res.results[j]["out"]).reshape(T_FULL, BL, A)
            for j in range(NCORES)]
    return np.concatenate(outs, axis=1)

